# revision 19
# baseline (speedup 1.0000x reference)
"""Trainium2 Bass kernel for the ActiveMemoryTensor forward pass.

Data-parallel over batch across 8 NeuronCores (64 batches/core, params
replicated). Per core:
  1. Bulk-copy memory -> new_memory with DRAM->DRAM DMAs split over the
     sync + scalar HWDGE queues (64MB of HBM traffic = the roofline).
  2. q = latent @ Wq + bq; logits = q @ Mk^T            (PE)
  3. top-32 of e=exp(logits-max) via max/match_replace  (DVE)
     masked renormalized weights (softmax denom cancels)
  4. indirect-DMA gather of the 32 selected slots/batch (GPSIMD, one
     offset per partition per instr - the HW contract), in "pair layout":
     pair n = b*32+k lives at partition n//16, tile-column n%16.
  5. read = sum_k w_k*mem[b,idx_k,:] via per-column weighting + strided
     reduce + a partition-pair-combining matmul.
  6. gate chain (sigmoid/tanh/cos projections), value   (PE/ACT/DVE)
  7. updated slots ns = m + gate*w*(v - m) computed directly in pair
     layout, indirect-DMA scattered into new_memory after the bulk copy.
Everything except the 64MB copy touches ~2MB, so the kernel runs at the
HBM streaming roofline plus the scatter tail.
"""

import numpy as np

B, H, S, D = 512, 1024, 1024, 128
TOPK = 32
NCORES = 8
BSH = B // NCORES  # 64
P = 128
C8 = S // P  # 8
NT = (BSH * TOPK) // P  # 16 gather/scatter tiles (one offset per partition)

_CACHE = {}


def _ensure_path():
    import sys

    for p in ("/opt/trn_rl_repo",):
        if p not in sys.path:
            sys.path.append(p)


def build_nc(n_copy=8):
    """Build the single-core SPMD program (same program on all 8 cores)."""
    _ensure_path()
    import math
    from contextlib import ExitStack

    import concourse.bacc as bacc
    import concourse.bass as bass
    import concourse.mybir as mybir
    from concourse.masks import make_identity
    from concourse.tile import TileContext, add_dep_helper

    f32 = mybir.dt.float32
    f32r = mybir.dt.float32r
    u32 = mybir.dt.uint32

    def R(ap):
        return ap
    AF = mybir.ActivationFunctionType
    OP = mybir.AluOpType
    AX = mybir.AxisListType

    nc = bacc.Bacc("TRN2", target_bir_lowering=False, num_devices=NCORES)

    lat_d = nc.dram_tensor("latent", [BSH, H], f32, kind="ExternalInput").ap()
    mem_d = nc.dram_tensor("memory", [BSH, S, D], f32, kind="ExternalInput").ap()
    rqw_d = nc.dram_tensor("read_query_w", [H, D], f32, kind="ExternalInput").ap()
    rqb_d = nc.dram_tensor("read_query_b", [D], f32, kind="ExternalInput").ap()
    mk_d = nc.dram_tensor("memory_key", [S, D], f32, kind="ExternalInput").ap()
    wgw_d = nc.dram_tensor("write_gate_w", [H + D, 1], f32, kind="ExternalInput").ap()
    wgb_d = nc.dram_tensor("write_gate_b", [1], f32, kind="ExternalInput").ap()
    dmw_d = nc.dram_tensor("dmd_proj_w", [H + D, 1], f32, kind="ExternalInput").ap()
    dmb_d = nc.dram_tensor("dmd_proj_b", [1], f32, kind="ExternalInput").ap()
    phw_d = nc.dram_tensor("phase_proj_w", [H, 1], f32, kind="ExternalInput").ap()
    phb_d = nc.dram_tensor("phase_proj_b", [1], f32, kind="ExternalInput").ap()
    wvw_d = nc.dram_tensor("write_value_w", [H, D], f32, kind="ExternalInput").ap()
    wvb_d = nc.dram_tensor("write_value_b", [D], f32, kind="ExternalInput").ap()

    read_d = nc.dram_tensor("read", [BSH, D], f32, kind="ExternalOutput").ap()
    newm_d = nc.dram_tensor("new_memory", [BSH, S, D], f32, kind="ExternalOutput").ap()
    wout_d = nc.dram_tensor("weights", [BSH, S], f32, kind="ExternalOutput").ap()

    HC = H // P  # 8 chunks over the H (contraction) dim
    mem_flat = mem_d.rearrange("b s d -> (b s) d")
    newm_flat = newm_d.rearrange("b s d -> (b s) d")

    with TileContext(nc) as tc, ExitStack() as stk:
        pro = stk.enter_context(tc.tile_pool(name="pro", bufs=1))
        pptr = stk.enter_context(tc.tile_pool(name="pptr", bufs=3, space="PSUM"))
        ppacc = stk.enter_context(tc.tile_pool(name="ppacc", bufs=1, space="PSUM"))
        ppl = stk.enter_context(tc.tile_pool(name="ppl", bufs=1, space="PSUM"))

        ident = pro.tile([P, P], f32)
        make_identity(nc, ident[:])
        ones1 = pro.tile([1, BSH], f32)
        nc.vector.memset(ones1[:], 1.0)

        # ---- param loads (ahead of the copies in the sync FIFO) ----
        wq_sb = pro.tile([P, HC * D], f32)
        nc.sync.dma_start(
            out=wq_sb[:].rearrange("p (c d) -> p c d", c=HC),
            in_=rqw_d.rearrange("(c p) d -> p c d", p=P),
        )
        mk_sb = pro.tile([P, C8 * D], f32)
        nc.sync.dma_start(
            out=mk_sb[:].rearrange("p (c d) -> p c d", c=C8),
            in_=mk_d.rearrange("(c p) d -> p c d", p=P),
        )
        lat = pro.tile([BSH, H], f32)
        nc.sync.dma_start(out=lat[:], in_=lat_d)
        wv_sb = pro.tile([P, HC * D], f32)
        nc.sync.dma_start(
            out=wv_sb[:].rearrange("p (c d) -> p c d", c=HC),
            in_=wvw_d.rearrange("(c p) d -> p c d", p=P),
        )
        bq_sb = pro.tile([1, D], f32)
        nc.sync.dma_start(out=bq_sb[:], in_=rqb_d.rearrange("(o d) -> o d", o=1))
        bv_sb = pro.tile([1, D], f32)
        nc.sync.dma_start(out=bv_sb[:], in_=wvb_d.rearrange("(o d) -> o d", o=1))
        wg_stage = pro.tile([3, H + D], f32)
        nc.vector.memset(wg_stage[:], 0.0)
        nc.sync.dma_start(out=wg_stage[0:1, :], in_=wgw_d.rearrange("k o -> o k"))
        nc.sync.dma_start(out=wg_stage[1:2, :], in_=dmw_d.rearrange("k o -> o k"))
        nc.sync.dma_start(out=wg_stage[2:3, :H], in_=phw_d.rearrange("k o -> o k"))
        b3_sb = pro.tile([1, 3], f32)
        nc.sync.dma_start(out=b3_sb[:, 0:1], in_=wgb_d.rearrange("(o k) -> o k", o=1))
        nc.sync.dma_start(out=b3_sb[:, 1:2], in_=dmb_d.rearrange("(o k) -> o k", o=1))
        nc.sync.dma_start(out=b3_sb[:, 2:3], in_=phb_d.rearrange("(o k) -> o k", o=1))

        # pair-combine matrices (gpsimd, independent of everything):
        # PC[p, b] = 1 iff p // 2 == b ; PCT = PC^T
        PC = pro.tile([P, BSH], f32)
        nc.gpsimd.memset(PC[:], 0.0)
        for base in (0, -1):
            nc.gpsimd.affine_select(
                out=PC[:], in_=PC[:], compare_op=OP.not_equal, fill=1.0,
                base=base, pattern=[[-2, BSH]], channel_multiplier=1,
            )
        PCT = pro.tile([BSH, P], f32)
        nc.gpsimd.memset(PCT[:], 0.0)
        for base in (0, -1):
            nc.gpsimd.affine_select(
                out=PCT[:], in_=PCT[:], compare_op=OP.not_equal, fill=1.0,
                base=base, pattern=[[1, P]], channel_multiplier=-2,
            )
        rowbase = pro.tile([BSH, 1], u32)
        nc.gpsimd.iota(rowbase[:], pattern=[[0, 1]], base=0, channel_multiplier=S)

        # ---- bulk copy memory -> new_memory (DRAM->DRAM, 2 HWDGE queues) ----
        copy_insts = []
        rows_per_copy = (BSH * S) // n_copy
        for i in range(n_copy):
            eng = nc.sync if i % 2 == 0 else nc.scalar
            ci = eng.dma_start(
                out=newm_flat[i * rows_per_copy : (i + 1) * rows_per_copy, :],
                in_=mem_flat[i * rows_per_copy : (i + 1) * rows_per_copy, :],
            )
            copy_insts.append(ci)

        # ---- critical chain to the top-k: mkT, latT, q, logits ----
        mkT = pro.tile([P, S], f32)
        for c in range(C8):
            tmk = pptr.tile([P, P], f32, tag="tr")
            nc.tensor.transpose(
                out=tmk[:], in_=mk_sb[:, c * D : (c + 1) * D], identity=ident[:]
            )
            nc.scalar.activation(out=mkT[:, c * P : (c + 1) * P], in_=tmk[:], func=AF.Copy)

        latT = pro.tile([P, HC * BSH], f32)
        for c in range(HC):
            tl = pptr.tile([P, BSH], f32, tag="tr")
            nc.tensor.transpose(
                out=tl[:], in_=lat[:, c * P : (c + 1) * P], identity=ident[:BSH, :BSH]
            )
            nc.scalar.activation(
                out=latT[:, c * BSH : (c + 1) * BSH], in_=tl[:], func=AF.Copy
            )

        q_ps = ppacc.tile([BSH, D], f32, tag="qps")
        for c in range(HC):
            nc.tensor.matmul(
                out=q_ps[:],
                lhsT=R(latT[:, c * BSH : (c + 1) * BSH]),
                rhs=R(wq_sb[:, c * D : (c + 1) * D]),
                start=(c == 0),
                stop=False,
            )
        nc.tensor.matmul(
            out=q_ps[:], lhsT=R(ones1[:]), rhs=R(bq_sb[:]), start=False, stop=True
        )
        q_sb = pro.tile([BSH, D], f32)
        nc.scalar.activation(out=q_sb[:], in_=q_ps[:], func=AF.Copy)
        qT_ps = ppacc.tile([P, BSH], f32, tag="qT")
        nc.tensor.transpose(out=qT_ps[:], in_=q_sb[:], identity=ident[:BSH, :BSH])
        qT = pro.tile([P, BSH], f32)
        nc.scalar.activation(out=qT[:], in_=qT_ps[:], func=AF.Copy)

        lg_ps = ppl.tile([BSH, S], f32)
        for hfi in range(2):
            nc.tensor.matmul(
                out=lg_ps[:, hfi * 512 : (hfi + 1) * 512],
                lhsT=R(qT[:]),
                rhs=R(mkT[:, hfi * 512 : (hfi + 1) * 512]),
                start=True,
                stop=True,
            )

        # ---- softmax numerator + top-32 ----
        negmax = pro.tile([BSH, 1], f32)
        nc.vector.tensor_reduce(
            out=negmax[:], in_=lg_ps[:], axis=AX.X, op=OP.max, negate=True
        )
        e_sb = pro.tile([BSH, S], f32)
        nc.scalar.activation(
            out=e_sb[:], in_=lg_ps[:], func=AF.Exp, bias=negmax[:], scale=1.0
        )

        m8 = pro.tile([BSH, TOPK], f32)
        i32t = pro.tile([BSH, TOPK], u32)
        zap_a = pro.tile([BSH, S], f32)
        zap_b = pro.tile([BSH, S], f32)
        cur = e_sb
        for r in range(TOPK // 8):
            sl = slice(r * 8, (r + 1) * 8)
            nc.vector.max(out=m8[:, sl], in_=cur[:])
            nc.vector.max_index(out=i32t[:, sl], in_max=m8[:, sl], in_values=cur[:])
            nxt = zap_a if (r % 2 == 0) else zap_b
            nc.vector.match_replace(
                out=nxt[:], in_to_replace=m8[:, sl], in_values=cur[:], imm_value=0.0
            )
            cur = nxt

        me = pro.tile([BSH, S], f32)
        nc.vector.tensor_sub(out=me[:], in0=e_sb[:], in1=cur[:])
        denom = pro.tile([BSH, 1], f32)
        nc.vector.tensor_reduce(out=denom[:], in_=me[:], axis=AX.X, op=OP.add)
        rcp = pro.tile([BSH, 1], f32)
        nc.vector.reciprocal(out=rcp[:], in_=denom[:])
        wtop = pro.tile([BSH, TOPK], f32)
        nc.vector.tensor_scalar_mul(wtop[:], m8[:], rcp[:])

        # global slot ids + pair-layout repartition ([64,32] -> [128,16])
        gidx = pro.tile([BSH, TOPK], u32)
        nc.vector.tensor_tensor(
            out=gidx[:], in0=i32t[:], in1=rowbase[:].to_broadcast([BSH, TOPK]), op=OP.add
        )
        gidx2 = pro.tile([P, NT], u32)
        idx_rep = nc.gpsimd.dma_start(out=gidx2[:], in_=gidx[:])
        G2 = pro.tile([P, NT * D], f32)
        for t in range(NT):
            gi = nc.gpsimd.indirect_dma_start(
                out=G2[:, t * D : (t + 1) * D],
                out_offset=None,
                in_=mem_flat,
                in_offset=bass.IndirectOffsetOnAxis(ap=gidx2[:, t : t + 1], axis=0),
                bounds_check=BSH * S - 1,
                oob_is_err=False,
            )
            add_dep_helper(gi.ins, idx_rep.ins, reason="gather offsets ready")

        # weights output (big but off the critical path; own DVE op + store)
        wnorm = pro.tile([BSH, S], f32)
        nc.vector.tensor_scalar_mul(wnorm[:], me[:], rcp[:])

        # pair-layout copies of the top weights (ahead of the gathers in
        # the SWDGE FIFO; ready as soon as wtop is)
        w_pairs = pro.tile([P, NT], f32)
        nc.gpsimd.dma_start(out=w_pairs[:], in_=wtop[:])
        nc.gpsimd.dma_start(out=wout_d, in_=wnorm[:])

        # ---- PE/ACT work that is independent of `read`, done while the
        # gathers run: gate lat-part (+bias), phase cos, value, V2 ----
        GC = (H + D) // P  # 9 gate-weight chunks
        wg3 = pro.tile([P, GC * 3], f32)
        for c in range(GC):
            t3 = pptr.tile([P, 3], f32, tag="tr")
            nc.tensor.transpose(
                out=t3[:], in_=wg_stage[:, c * P : (c + 1) * P], identity=ident[:3, :3]
            )
            nc.scalar.activation(out=wg3[:, c * 3 : (c + 1) * 3], in_=t3[:], func=AF.Copy)

        g3_ps = ppacc.tile([BSH, 3], f32, tag="g3")
        for c in range(HC):
            nc.tensor.matmul(
                out=g3_ps[:],
                lhsT=R(latT[:, c * BSH : (c + 1) * BSH]),
                rhs=R(wg3[:, c * 3 : (c + 1) * 3]),
                start=(c == 0),
                stop=False,
            )
        nc.tensor.matmul(
            out=g3_ps[:], lhsT=R(ones1[:]), rhs=R(b3_sb[:]), start=False, stop=True
        )
        # phase gating factor (col 2 gets no read contribution: Wp has no
        # read rows) - compute while the gathers run
        halfpi = pro.tile([BSH, 1], f32)
        nc.vector.memset(halfpi[:], float(math.pi / 2))
        cosp = pro.tile([BSH, 1], f32)
        nc.scalar.activation(
            out=cosp[:], in_=g3_ps[:, 2:3], func=AF.Sin, scale=1.0, bias=halfpi[:]
        )
        c2 = pro.tile([BSH, 1], f32)
        nc.scalar.activation(out=c2[:], in_=cosp[:], func=AF.Copy, scale=0.5, bias=0.5)

        v_ps = ppacc.tile([BSH, D], f32, tag="qps")
        for c in range(HC):
            nc.tensor.matmul(
                out=v_ps[:],
                lhsT=R(latT[:, c * BSH : (c + 1) * BSH]),
                rhs=R(wv_sb[:, c * D : (c + 1) * D]),
                start=(c == 0),
                stop=False,
            )
        nc.tensor.matmul(
            out=v_ps[:], lhsT=R(ones1[:]), rhs=R(bv_sb[:]), start=False, stop=True
        )
        v_sb = pro.tile([BSH, D], f32)
        nc.scalar.activation(out=v_sb[:], in_=v_ps[:], func=AF.Copy)
        V2_ps = ppacc.tile([P, D], f32, tag="qT")
        nc.tensor.matmul(out=V2_ps[:], lhsT=R(PCT[:]), rhs=R(v_sb[:]), start=True, stop=True)
        V2 = pro.tile([P, D], f32)
        nc.scalar.activation(out=V2[:], in_=V2_ps[:], func=AF.Copy)
        # pre-warm the Sigmoid/Tanh activation tables off the critical path
        warm = pro.tile([1, 1], f32)
        nc.scalar.activation(out=warm[:], in_=ones1[:, 0:1], func=AF.Sigmoid)
        nc.scalar.activation(out=warm[:], in_=ones1[:, 0:1], func=AF.Tanh)

        # ---- read = sum_k wtop*G in pair layout: chained weighted
        # accumulation, one DVE op per gather chunk (pipelines with the
        # gathers). Also t1 = v - m per chunk, ready before the gate. ----
        R_a = pro.tile([P, D], f32)
        R_b = pro.tile([P, D], f32)
        t1 = pro.tile([P, NT * D], f32)
        nc.vector.tensor_scalar_mul(R_a[:], G2[:, 0:D], w_pairs[:, 0:1])
        nc.vector.tensor_sub(out=t1[:, 0:D], in0=V2[:], in1=G2[:, 0:D])
        for t in range(1, NT):
            sl = slice(t * D, (t + 1) * D)
            acc_in = R_a if t % 2 == 1 else R_b
            acc_out = R_b if t % 2 == 1 else R_a
            nc.vector.scalar_tensor_tensor(
                out=acc_out[:],
                in0=G2[:, sl],
                scalar=w_pairs[:, t : t + 1],
                in1=acc_in[:],
                op0=OP.mult,
                op1=OP.add,
            )
            nc.vector.tensor_sub(out=t1[:, sl], in0=V2[:], in1=G2[:, sl])
        R1 = R_b if (NT - 1) % 2 == 1 else R_a
        read_ps = ppacc.tile([BSH, D], f32, tag="qps")
        nc.tensor.matmul(out=read_ps[:], lhsT=R(PC[:]), rhs=R(R1[:]), start=True, stop=True)
        read_sb = pro.tile([BSH, D], f32)
        nc.scalar.activation(out=read_sb[:], in_=read_ps[:], func=AF.Copy)
        nc.gpsimd.dma_start(out=read_d, in_=read_sb[:])

        # ---- finish the gate with the read contribution (cols 0,1) ----
        rT_ps = ppacc.tile([P, BSH], f32, tag="qT")
        nc.tensor.transpose(out=rT_ps[:], in_=read_sb[:], identity=ident[:BSH, :BSH])
        rT = pro.tile([P, BSH], f32)
        nc.scalar.activation(out=rT[:], in_=rT_ps[:], func=AF.Copy)
        nc.tensor.matmul(
            out=g3_ps[:, 0:2], lhsT=R(rT[:]), rhs=R(wg3[:, HC * 3 : HC * 3 + 2]),
            start=False, stop=True, skip_group_check=True,
        )

        gsig = pro.tile([BSH, 1], f32)
        nc.scalar.activation(out=gsig[:], in_=g3_ps[:, 0:1], func=AF.Sigmoid)
        gtan = pro.tile([BSH, 1], f32)
        nc.scalar.activation(out=gtan[:], in_=g3_ps[:, 1:2], func=AF.Tanh)
        fac = pro.tile([BSH, 1], f32)
        nc.vector.tensor_scalar(
            out=fac[:], in0=gtan[:], scalar1=0.25, scalar2=1.0, op0=OP.mult, op1=OP.add
        )
        g1 = pro.tile([BSH, 1], f32)
        nc.vector.tensor_mul(g1[:], gsig[:], fac[:])
        nc.vector.tensor_scalar(
            out=g1[:], in0=g1[:], scalar1=0.0, scalar2=1.0, op0=OP.max, op1=OP.min
        )
        gate = pro.tile([BSH, 1], f32)
        nc.vector.tensor_mul(gate[:], g1[:], c2[:])

        # ---- updated slots in pair layout: ns = m + a*(v - m) ----
        a32 = pro.tile([BSH, TOPK], f32)
        nc.vector.tensor_scalar_mul(a32[:], wtop[:], gate[:])
        a_pairs = pro.tile([P, NT], f32)
        nc.gpsimd.dma_start(out=a_pairs[:], in_=a32[:])
        ns2 = pro.tile([P, NT * D], f32)
        for t in range(NT):
            sl = slice(t * D, (t + 1) * D)
            nc.vector.scalar_tensor_tensor(
                out=ns2[:, sl],
                in0=t1[:, sl],
                scalar=a_pairs[:, t : t + 1],
                in1=G2[:, sl],
                op0=OP.mult,
                op1=OP.add,
            )

        # ---- scatter the updated slots (after the bulk copy) ----
        for t in range(NT):
            sc = nc.gpsimd.indirect_dma_start(
                out=newm_flat,
                out_offset=bass.IndirectOffsetOnAxis(ap=gidx2[:, t : t + 1], axis=0),
                in_=ns2[:, t * D : (t + 1) * D],
                in_offset=None,
                bounds_check=BSH * S - 1,
                oob_is_err=False,
            )
            add_dep_helper(sc.ins, idx_rep.ins, reason="scatter offsets ready")
            for ci in copy_insts:
                add_dep_helper(
                    sc.ins, ci.ins, reason="scatter updated slots after bulk copy"
                )

    nc.compile()
    return nc


def _shard_inputs(inputs):
    arrs = {
        k: np.ascontiguousarray(np.asarray(v, dtype=np.float32))
        for k, v in inputs.items()
    }
    in_maps = []
    for i in range(NCORES):
        sl = slice(i * BSH, (i + 1) * BSH)
        m = {}
        for k, v in arrs.items():
            if k in ("latent", "memory"):
                m[k] = np.ascontiguousarray(v[sl])
            else:
                m[k] = v
        in_maps.append(m)
    return in_maps


def kernel(**inputs):
    _ensure_path()
    from concourse.bass_utils import run_bass_kernel_spmd

    nc = _CACHE.get("nc")
    if nc is None:
        nc = build_nc()
        _CACHE["nc"] = nc

    in_maps = _shard_inputs(inputs)
    res = run_bass_kernel_spmd(nc, in_maps, core_ids=list(range(NCORES)))
    read = np.concatenate([res.results[i]["read"] for i in range(NCORES)], axis=0)
    new_memory = np.concatenate(
        [res.results[i]["new_memory"] for i in range(NCORES)], axis=0
    )
    weights = np.concatenate([res.results[i]["weights"] for i in range(NCORES)], axis=0)
    return read, new_memory, weights


# revision 20
# speedup vs baseline: 1.1912x; 1.1912x over previous
"""Trainium2 Bass kernel for the ActiveMemoryTensor forward pass.

Data-parallel over batch across 8 NeuronCores (64 batches/core, params
replicated). Per core:
  1. Bulk-copy memory -> new_memory with DRAM->DRAM DMAs split over the
     sync + scalar HWDGE queues (64MB of HBM traffic = the roofline).
  2. q = latent @ Wq + bq; logits = q @ Mk^T            (PE)
  3. top-32 of e=exp(logits-max) via max/match_replace  (DVE)
     masked renormalized weights (softmax denom cancels)
  4. indirect-DMA gather of the 32 selected slots/batch (GPSIMD, one
     offset per partition per instr - the HW contract), in "pair layout":
     pair n = b*32+k lives at partition n//16, tile-column n%16.
  5. read = sum_k w_k*mem[b,idx_k,:] via per-column weighting + strided
     reduce + a partition-pair-combining matmul.
  6. gate chain (sigmoid/tanh/cos projections), value   (PE/ACT/DVE)
  7. updated slots ns = m + gate*w*(v - m) computed directly in pair
     layout, indirect-DMA scattered into new_memory after the bulk copy.
Everything except the 64MB copy touches ~2MB, so the kernel runs at the
HBM streaming roofline plus the scatter tail.
"""

import numpy as np

B, H, S, D = 512, 1024, 1024, 128
TOPK = 32
NCORES = 8
BSH = B // NCORES  # 64
P = 128
C8 = S // P  # 8
NT = (BSH * TOPK) // P  # 16 gather/scatter tiles (one offset per partition)

_CACHE = {}


def _ensure_path():
    import sys

    for p in ("/opt/trn_rl_repo",):
        if p not in sys.path:
            sys.path.append(p)


def build_nc(n_copy=16):
    """Build the single-core SPMD program (same program on all 8 cores)."""
    _ensure_path()
    import math
    from contextlib import ExitStack

    import concourse.bacc as bacc
    import concourse.bass as bass
    import concourse.mybir as mybir
    from concourse.masks import make_identity
    from concourse.tile import TileContext, add_dep_helper

    f32 = mybir.dt.float32
    f32r = mybir.dt.float32r
    u32 = mybir.dt.uint32

    def R(ap):
        return ap
    AF = mybir.ActivationFunctionType
    OP = mybir.AluOpType
    AX = mybir.AxisListType

    nc = bacc.Bacc("TRN2", target_bir_lowering=False, num_devices=NCORES)

    lat_d = nc.dram_tensor("latent", [BSH, H], f32, kind="ExternalInput").ap()
    mem_d = nc.dram_tensor("memory", [BSH, S, D], f32, kind="ExternalInput").ap()
    rqw_d = nc.dram_tensor("read_query_w", [H, D], f32, kind="ExternalInput").ap()
    rqb_d = nc.dram_tensor("read_query_b", [D], f32, kind="ExternalInput").ap()
    mk_d = nc.dram_tensor("memory_key", [S, D], f32, kind="ExternalInput").ap()
    wgw_d = nc.dram_tensor("write_gate_w", [H + D, 1], f32, kind="ExternalInput").ap()
    wgb_d = nc.dram_tensor("write_gate_b", [1], f32, kind="ExternalInput").ap()
    dmw_d = nc.dram_tensor("dmd_proj_w", [H + D, 1], f32, kind="ExternalInput").ap()
    dmb_d = nc.dram_tensor("dmd_proj_b", [1], f32, kind="ExternalInput").ap()
    phw_d = nc.dram_tensor("phase_proj_w", [H, 1], f32, kind="ExternalInput").ap()
    phb_d = nc.dram_tensor("phase_proj_b", [1], f32, kind="ExternalInput").ap()
    wvw_d = nc.dram_tensor("write_value_w", [H, D], f32, kind="ExternalInput").ap()
    wvb_d = nc.dram_tensor("write_value_b", [D], f32, kind="ExternalInput").ap()

    read_d = nc.dram_tensor("read", [BSH, D], f32, kind="ExternalOutput").ap()
    newm_d = nc.dram_tensor("new_memory", [BSH, S, D], f32, kind="ExternalOutput").ap()
    wout_d = nc.dram_tensor("weights", [BSH, S], f32, kind="ExternalOutput").ap()

    HC = H // P  # 8 chunks over the H (contraction) dim
    mem_flat = mem_d.rearrange("b s d -> (b s) d")
    newm_flat = newm_d.rearrange("b s d -> (b s) d")

    with TileContext(nc) as tc, ExitStack() as stk:
        pro = stk.enter_context(tc.tile_pool(name="pro", bufs=1))
        pptr = stk.enter_context(tc.tile_pool(name="pptr", bufs=3, space="PSUM"))
        ppacc = stk.enter_context(tc.tile_pool(name="ppacc", bufs=1, space="PSUM"))
        ppl = stk.enter_context(tc.tile_pool(name="ppl", bufs=1, space="PSUM"))

        ident = pro.tile([P, P], f32)
        make_identity(nc, ident[:])
        ones1 = pro.tile([1, BSH], f32)
        nc.vector.memset(ones1[:], 1.0)

        # ---- param loads (ahead of the copies in the sync FIFO) ----
        wq_sb = pro.tile([P, HC * D], f32)
        nc.sync.dma_start(
            out=wq_sb[:].rearrange("p (c d) -> p c d", c=HC),
            in_=rqw_d.rearrange("(c p) d -> p c d", p=P),
        )
        mk_sb = pro.tile([P, C8 * D], f32)
        nc.sync.dma_start(
            out=mk_sb[:].rearrange("p (c d) -> p c d", c=C8),
            in_=mk_d.rearrange("(c p) d -> p c d", p=P),
        )
        lat = pro.tile([BSH, H], f32)
        nc.sync.dma_start(out=lat[:], in_=lat_d)
        wv_sb = pro.tile([P, HC * D], f32)
        nc.sync.dma_start(
            out=wv_sb[:].rearrange("p (c d) -> p c d", c=HC),
            in_=wvw_d.rearrange("(c p) d -> p c d", p=P),
        )
        bq_sb = pro.tile([1, D], f32)
        nc.sync.dma_start(out=bq_sb[:], in_=rqb_d.rearrange("(o d) -> o d", o=1))
        bv_sb = pro.tile([1, D], f32)
        nc.sync.dma_start(out=bv_sb[:], in_=wvb_d.rearrange("(o d) -> o d", o=1))
        wg_stage = pro.tile([3, H + D], f32)
        nc.vector.memset(wg_stage[:], 0.0)
        nc.sync.dma_start(out=wg_stage[0:1, :], in_=wgw_d.rearrange("k o -> o k"))
        nc.sync.dma_start(out=wg_stage[1:2, :], in_=dmw_d.rearrange("k o -> o k"))
        nc.sync.dma_start(out=wg_stage[2:3, :H], in_=phw_d.rearrange("k o -> o k"))
        b3_sb = pro.tile([1, 3], f32)
        nc.sync.dma_start(out=b3_sb[:, 0:1], in_=wgb_d.rearrange("(o k) -> o k", o=1))
        nc.sync.dma_start(out=b3_sb[:, 1:2], in_=dmb_d.rearrange("(o k) -> o k", o=1))
        nc.sync.dma_start(out=b3_sb[:, 2:3], in_=phb_d.rearrange("(o k) -> o k", o=1))

        # pair-combine matrices (gpsimd, independent of everything):
        # PC[p, b] = 1 iff p // 2 == b ; PCT = PC^T
        PC = pro.tile([P, BSH], f32)
        nc.gpsimd.memset(PC[:], 0.0)
        for base in (0, -1):
            nc.gpsimd.affine_select(
                out=PC[:], in_=PC[:], compare_op=OP.not_equal, fill=1.0,
                base=base, pattern=[[-2, BSH]], channel_multiplier=1,
            )
        PCT = pro.tile([BSH, P], f32)
        nc.gpsimd.memset(PCT[:], 0.0)
        for base in (0, -1):
            nc.gpsimd.affine_select(
                out=PCT[:], in_=PCT[:], compare_op=OP.not_equal, fill=1.0,
                base=base, pattern=[[1, P]], channel_multiplier=-2,
            )
        rowbase = pro.tile([BSH, 1], u32)
        nc.gpsimd.iota(rowbase[:], pattern=[[0, 1]], base=0, channel_multiplier=S)

        # ---- bulk copy memory -> new_memory (DRAM->DRAM, 2 HWDGE queues) ----
        copy_insts = []
        rows_per_copy = (BSH * S) // n_copy
        for i in range(n_copy):
            eng = nc.sync if i < (7 * n_copy) // 16 else nc.scalar
            ci = eng.dma_start(
                out=newm_flat[i * rows_per_copy : (i + 1) * rows_per_copy, :],
                in_=mem_flat[i * rows_per_copy : (i + 1) * rows_per_copy, :],
            )
            copy_insts.append(ci)

        # ---- critical chain to the top-k: mkT, latT, q, logits ----
        mkT = pro.tile([P, S], f32)
        for c in range(C8):
            tmk = pptr.tile([P, P], f32, tag="tr")
            nc.tensor.transpose(
                out=tmk[:], in_=mk_sb[:, c * D : (c + 1) * D], identity=ident[:]
            )
            nc.scalar.activation(out=mkT[:, c * P : (c + 1) * P], in_=tmk[:], func=AF.Copy)

        latT = pro.tile([P, HC * BSH], f32)
        for c in range(HC):
            tl = pptr.tile([P, BSH], f32, tag="tr")
            nc.tensor.transpose(
                out=tl[:], in_=lat[:, c * P : (c + 1) * P], identity=ident[:BSH, :BSH]
            )
            nc.scalar.activation(
                out=latT[:, c * BSH : (c + 1) * BSH], in_=tl[:], func=AF.Copy
            )

        q_ps = ppacc.tile([BSH, D], f32, tag="qps")
        for c in range(HC):
            nc.tensor.matmul(
                out=q_ps[:],
                lhsT=R(latT[:, c * BSH : (c + 1) * BSH]),
                rhs=R(wq_sb[:, c * D : (c + 1) * D]),
                start=(c == 0),
                stop=False,
            )
        nc.tensor.matmul(
            out=q_ps[:], lhsT=R(ones1[:]), rhs=R(bq_sb[:]), start=False, stop=True
        )
        q_sb = pro.tile([BSH, D], f32)
        nc.scalar.activation(out=q_sb[:], in_=q_ps[:], func=AF.Copy)
        qT_ps = ppacc.tile([P, BSH], f32, tag="qT")
        nc.tensor.transpose(out=qT_ps[:], in_=q_sb[:], identity=ident[:BSH, :BSH])
        qT = pro.tile([P, BSH], f32)
        nc.scalar.activation(out=qT[:], in_=qT_ps[:], func=AF.Copy)

        lg_ps = ppl.tile([BSH, S], f32)
        for hfi in range(2):
            nc.tensor.matmul(
                out=lg_ps[:, hfi * 512 : (hfi + 1) * 512],
                lhsT=R(qT[:]),
                rhs=R(mkT[:, hfi * 512 : (hfi + 1) * 512]),
                start=True,
                stop=True,
            )

        # ---- top-32 straight on the PSUM logits (exp is off this path;
        # the renorm denominator only needs the 32 top exps) ----
        negmax = pro.tile([BSH, 1], f32)
        nc.vector.tensor_reduce(
            out=negmax[:], in_=lg_ps[:], axis=AX.X, op=OP.max, negate=True
        )
        m8 = pro.tile([BSH, TOPK], f32)  # top-32 logits, descending per round
        i32t = pro.tile([BSH, TOPK], u32)
        zap_a = pro.tile([BSH, S], f32)
        zap_b = pro.tile([BSH, S], f32)
        NEG = -1.0e30
        cur = lg_ps
        for r in range(TOPK // 8):
            sl = slice(r * 8, (r + 1) * 8)
            nc.vector.max(out=m8[:, sl], in_=cur[:])
            nc.vector.max_index(out=i32t[:, sl], in_max=m8[:, sl], in_values=cur[:])
            nxt = zap_a if (r % 2 == 0) else zap_b
            nc.vector.match_replace(
                out=nxt[:], in_to_replace=m8[:, sl], in_values=cur[:], imm_value=NEG
            )
            cur = nxt

        e_top = pro.tile([BSH, TOPK], f32)
        nc.scalar.activation(
            out=e_top[:], in_=m8[:], func=AF.Exp, bias=negmax[:], scale=1.0
        )
        denom = pro.tile([BSH, 1], f32)
        nc.vector.tensor_reduce(out=denom[:], in_=e_top[:], axis=AX.X, op=OP.add)
        rcp = pro.tile([BSH, 1], f32)
        nc.vector.reciprocal(out=rcp[:], in_=denom[:])
        wtop = pro.tile([BSH, TOPK], f32)
        nc.vector.tensor_scalar_mul(wtop[:], e_top[:], rcp[:])

        # global slot ids + pair-layout repartition ([64,32] -> [128,16])
        gidx = pro.tile([BSH, TOPK], u32)
        nc.vector.tensor_tensor(
            out=gidx[:], in0=i32t[:], in1=rowbase[:].to_broadcast([BSH, TOPK]), op=OP.add
        )
        gidx2 = pro.tile([P, NT], u32)
        idx_rep = nc.gpsimd.dma_start(out=gidx2[:], in_=gidx[:])
        G2 = pro.tile([P, NT * D], f32)
        for t in range(NT):
            gi = nc.gpsimd.indirect_dma_start(
                out=G2[:, t * D : (t + 1) * D],
                out_offset=None,
                in_=mem_flat,
                in_offset=bass.IndirectOffsetOnAxis(ap=gidx2[:, t : t + 1], axis=0),
                bounds_check=BSH * S - 1,
                oob_is_err=False,
            )
            add_dep_helper(gi.ins, idx_rep.ins, reason="gather offsets ready")

        # weights output (big but fully off the critical path):
        # w = (exp(logits-max) - exp(zapped-max)) / denom; zapped top-32
        # are -1e30 so their exp underflows to 0.
        e_sb = pro.tile([BSH, S], f32)
        nc.scalar.activation(
            out=e_sb[:], in_=lg_ps[:], func=AF.Exp, bias=negmax[:], scale=1.0
        )
        e_zap = pro.tile([BSH, S], f32)
        nc.scalar.activation(
            out=e_zap[:], in_=cur[:], func=AF.Exp, bias=negmax[:], scale=1.0
        )
        me = pro.tile([BSH, S], f32)
        nc.vector.tensor_sub(out=me[:], in0=e_sb[:], in1=e_zap[:])
        wnorm = pro.tile([BSH, S], f32)
        nc.vector.tensor_scalar_mul(wnorm[:], me[:], rcp[:])

        # pair-layout copies of the top weights (ahead of the gathers in
        # the SWDGE FIFO; ready as soon as wtop is)
        w_pairs = pro.tile([P, NT], f32)
        nc.gpsimd.dma_start(out=w_pairs[:], in_=wtop[:])
        nc.gpsimd.dma_start(out=wout_d, in_=wnorm[:])

        # ---- PE/ACT work that is independent of `read`, done while the
        # gathers run: gate lat-part (+bias), phase cos, value, V2 ----
        GC = (H + D) // P  # 9 gate-weight chunks
        wg3 = pro.tile([P, GC * 3], f32)
        for c in range(GC):
            t3 = pptr.tile([P, 3], f32, tag="tr")
            nc.tensor.transpose(
                out=t3[:], in_=wg_stage[:, c * P : (c + 1) * P], identity=ident[:3, :3]
            )
            nc.scalar.activation(out=wg3[:, c * 3 : (c + 1) * 3], in_=t3[:], func=AF.Copy)

        g3_ps = ppacc.tile([BSH, 3], f32, tag="g3")
        for c in range(HC):
            nc.tensor.matmul(
                out=g3_ps[:],
                lhsT=R(latT[:, c * BSH : (c + 1) * BSH]),
                rhs=R(wg3[:, c * 3 : (c + 1) * 3]),
                start=(c == 0),
                stop=False,
            )
        nc.tensor.matmul(
            out=g3_ps[:], lhsT=R(ones1[:]), rhs=R(b3_sb[:]), start=False, stop=True
        )
        # phase gating factor (col 2 gets no read contribution: Wp has no
        # read rows) - compute while the gathers run
        halfpi = pro.tile([BSH, 1], f32)
        nc.vector.memset(halfpi[:], float(math.pi / 2))
        cosp = pro.tile([BSH, 1], f32)
        nc.scalar.activation(
            out=cosp[:], in_=g3_ps[:, 2:3], func=AF.Sin, scale=1.0, bias=halfpi[:]
        )
        c2 = pro.tile([BSH, 1], f32)
        nc.scalar.activation(out=c2[:], in_=cosp[:], func=AF.Copy, scale=0.5, bias=0.5)

        v_ps = ppacc.tile([BSH, D], f32, tag="qps")
        for c in range(HC):
            nc.tensor.matmul(
                out=v_ps[:],
                lhsT=R(latT[:, c * BSH : (c + 1) * BSH]),
                rhs=R(wv_sb[:, c * D : (c + 1) * D]),
                start=(c == 0),
                stop=False,
            )
        nc.tensor.matmul(
            out=v_ps[:], lhsT=R(ones1[:]), rhs=R(bv_sb[:]), start=False, stop=True
        )
        v_sb = pro.tile([BSH, D], f32)
        nc.scalar.activation(out=v_sb[:], in_=v_ps[:], func=AF.Copy)
        V2_ps = ppacc.tile([P, D], f32, tag="qT")
        nc.tensor.matmul(out=V2_ps[:], lhsT=R(PCT[:]), rhs=R(v_sb[:]), start=True, stop=True)
        V2 = pro.tile([P, D], f32)
        nc.scalar.activation(out=V2[:], in_=V2_ps[:], func=AF.Copy)
        # pre-warm the Sigmoid/Tanh activation tables off the critical path
        warm = pro.tile([1, 1], f32)
        nc.scalar.activation(out=warm[:], in_=ones1[:, 0:1], func=AF.Sigmoid)
        nc.scalar.activation(out=warm[:], in_=ones1[:, 0:1], func=AF.Tanh)

        # ---- read = sum_k wtop*G in pair layout: chained weighted
        # accumulation, one DVE op per gather chunk (pipelines with the
        # gathers). Also t1 = v - m per chunk, ready before the gate. ----
        R_a = pro.tile([P, D], f32)
        R_b = pro.tile([P, D], f32)
        t1 = pro.tile([P, NT * D], f32)
        nc.vector.tensor_scalar_mul(R_a[:], G2[:, 0:D], w_pairs[:, 0:1])
        nc.vector.tensor_sub(out=t1[:, 0:D], in0=V2[:], in1=G2[:, 0:D])
        for t in range(1, NT):
            sl = slice(t * D, (t + 1) * D)
            acc_in = R_a if t % 2 == 1 else R_b
            acc_out = R_b if t % 2 == 1 else R_a
            nc.vector.scalar_tensor_tensor(
                out=acc_out[:],
                in0=G2[:, sl],
                scalar=w_pairs[:, t : t + 1],
                in1=acc_in[:],
                op0=OP.mult,
                op1=OP.add,
            )
            nc.vector.tensor_sub(out=t1[:, sl], in0=V2[:], in1=G2[:, sl])
        R1 = R_b if (NT - 1) % 2 == 1 else R_a
        read_ps = ppacc.tile([BSH, D], f32, tag="qps")
        nc.tensor.matmul(out=read_ps[:], lhsT=R(PC[:]), rhs=R(R1[:]), start=True, stop=True)
        read_sb = pro.tile([BSH, D], f32)
        nc.scalar.activation(out=read_sb[:], in_=read_ps[:], func=AF.Copy)
        nc.gpsimd.dma_start(out=read_d, in_=read_sb[:])

        # ---- finish the gate with the read contribution (cols 0,1) ----
        rT_ps = ppacc.tile([P, BSH], f32, tag="qT")
        nc.tensor.transpose(out=rT_ps[:], in_=read_sb[:], identity=ident[:BSH, :BSH])
        rT = pro.tile([P, BSH], f32)
        nc.scalar.activation(out=rT[:], in_=rT_ps[:], func=AF.Copy)
        nc.tensor.matmul(
            out=g3_ps[:, 0:2], lhsT=R(rT[:]), rhs=R(wg3[:, HC * 3 : HC * 3 + 2]),
            start=False, stop=True, skip_group_check=True,
        )

        gsig = pro.tile([BSH, 1], f32)
        nc.scalar.activation(out=gsig[:], in_=g3_ps[:, 0:1], func=AF.Sigmoid)
        gtan = pro.tile([BSH, 1], f32)
        nc.scalar.activation(out=gtan[:], in_=g3_ps[:, 1:2], func=AF.Tanh)
        fac = pro.tile([BSH, 1], f32)
        nc.vector.tensor_scalar(
            out=fac[:], in0=gtan[:], scalar1=0.25, scalar2=1.0, op0=OP.mult, op1=OP.add
        )
        g1 = pro.tile([BSH, 1], f32)
        nc.vector.tensor_mul(g1[:], gsig[:], fac[:])
        nc.vector.tensor_scalar(
            out=g1[:], in0=g1[:], scalar1=0.0, scalar2=1.0, op0=OP.max, op1=OP.min
        )
        gate = pro.tile([BSH, 1], f32)
        nc.vector.tensor_mul(gate[:], g1[:], c2[:])

        # ---- updated slots in pair layout: ns = m + a*(v - m) ----
        a32 = pro.tile([BSH, TOPK], f32)
        nc.vector.tensor_scalar_mul(a32[:], wtop[:], gate[:])
        a_pairs = pro.tile([P, NT], f32)
        nc.gpsimd.dma_start(out=a_pairs[:], in_=a32[:])
        ns2 = pro.tile([P, NT * D], f32)
        for t in range(NT):
            sl = slice(t * D, (t + 1) * D)
            nc.vector.scalar_tensor_tensor(
                out=ns2[:, sl],
                in0=t1[:, sl],
                scalar=a_pairs[:, t : t + 1],
                in1=G2[:, sl],
                op0=OP.mult,
                op1=OP.add,
            )

        # ---- scatter the updated slots (after the bulk copy). The
        # scatters write disjoint rows, so the tensor-level WAW chain Tile
        # adds between them is false - demote it to no-sync so descriptor
        # generation of scatter t+1 overlaps the drain of scatter t. ----
        prev_scatters = []
        for t in range(NT):
            sc = nc.gpsimd.indirect_dma_start(
                out=newm_flat,
                out_offset=bass.IndirectOffsetOnAxis(ap=gidx2[:, t : t + 1], axis=0),
                in_=ns2[:, t * D : (t + 1) * D],
                in_offset=None,
                bounds_check=BSH * S - 1,
                oob_is_err=False,
            )
            for prev in prev_scatters:
                if sc.ins.try_remove_dependency(prev.ins.name):
                    add_dep_helper(sc.ins, prev.ins, sync=False, reason="demoted WAW")
            prev_scatters.append(sc)
            add_dep_helper(sc.ins, idx_rep.ins, reason="scatter offsets ready")
            for ci in copy_insts:
                add_dep_helper(
                    sc.ins, ci.ins, reason="scatter updated slots after bulk copy"
                )

    nc.compile()
    return nc


def _shard_inputs(inputs):
    arrs = {
        k: np.ascontiguousarray(np.asarray(v, dtype=np.float32))
        for k, v in inputs.items()
    }
    in_maps = []
    for i in range(NCORES):
        sl = slice(i * BSH, (i + 1) * BSH)
        m = {}
        for k, v in arrs.items():
            if k in ("latent", "memory"):
                m[k] = np.ascontiguousarray(v[sl])
            else:
                m[k] = v
        in_maps.append(m)
    return in_maps


def kernel(**inputs):
    _ensure_path()
    from concourse.bass_utils import run_bass_kernel_spmd

    nc = _CACHE.get("nc")
    if nc is None:
        nc = build_nc()
        _CACHE["nc"] = nc

    in_maps = _shard_inputs(inputs)
    res = run_bass_kernel_spmd(nc, in_maps, core_ids=list(range(NCORES)))
    read = np.concatenate([res.results[i]["read"] for i in range(NCORES)], axis=0)
    new_memory = np.concatenate(
        [res.results[i]["new_memory"] for i in range(NCORES)], axis=0
    )
    weights = np.concatenate([res.results[i]["weights"] for i in range(NCORES)], axis=0)
    return read, new_memory, weights
